# revision 30
# baseline (speedup 1.0000x reference)
"""Causal multi-head attention layer (train forward) on 8 Trainium2 NeuronCores.

Sharding: batch (4) x head-group (2 of 8 heads each) -> 8 cores.

Schedule: the scalar (ACT) engine is the only engine with Exp, so its softmax
exp stream (~162us) is kept exclusive and the PE stream is emission-interleaved
against it: projection / output-projection matmuls are pulled from a filler
queue between attention chunks so the PE never idles (and never drops out of
its high p-state) while ACT chews exps.  Drains run on DVE or straight DMA
(rowsums + ctx staging DMA directly out of PSUM), normalization uses the
single-op reciprocal_approx_fast.  Host pre-casts weights/x to bf16, sums the
two partials per batch, adds bo.
"""
import numpy as np
import ml_dtypes
from collections import deque

import concourse.bass as bass
import concourse.tile as tile
from concourse import bacc, mybir
from concourse.bass_utils import run_bass_kernel_spmd

F32 = mybir.dt.float32
BF16 = mybir.dt.bfloat16
AF = mybir.ActivationFunctionType
ALU = mybir.AluOpType

P = 128
D = 1024          # model dim
DC = 512          # per-core head dims (8 heads x 64)
HD = 64
NHC = 8           # heads per core
NPAIR = 4         # head pairs per core
FC = D // P       # 8 feature chunks
OC = DC // P      # 4 outdim chunks (= head pairs)
W = 512           # query window (fp32 PSUM bank)
WT = W // P       # token chunks per window
SCALE = 1.0 / 32.0  # 1/sqrt(D)


def build_nc(S=2048, num_devices=8, with_bv=False):
    NWIN = S // W

    nc = bacc.Bacc("TRN2", target_bir_lowering=False, debug=False,
                   num_devices=num_devices)
    xt = nc.dram_tensor("xt", [P, FC, S], BF16, kind="ExternalInput").ap()
    wq = nc.dram_tensor("wq", [P, FC, DC], BF16, kind="ExternalInput").ap()
    wk = nc.dram_tensor("wk", [P, FC, DC], BF16, kind="ExternalInput").ap()
    wv = nc.dram_tensor("wv", [P, FC, DC], BF16, kind="ExternalInput").ap()
    wo = nc.dram_tensor("wo", [P, OC, D], BF16, kind="ExternalInput").ap()
    bias3 = nc.dram_tensor("bias3", [P, 4 * OC], F32,
                           kind="ExternalInput").ap()
    tri = nc.dram_tensor("tri", [P, P], BF16, kind="ExternalInput").ap()
    out = nc.dram_tensor("out", [S, D], F32, kind="ExternalOutput").ap()

    with tile.TileContext(nc) as tc:
        with tc.tile_pool(name="const", bufs=1) as cst, \
             tc.tile_pool(name="stage", bufs=2) as stg, \
             tc.tile_pool(name="pt", bufs=3) as ptp, \
             tc.tile_pool(name="small", bufs=2) as sml, \
             tc.tile_pool(name="psS", bufs=1, space="PSUM") as psS, \
             tc.tile_pool(name="psC", bufs=1, space="PSUM") as psC:

            # --- PSUM strips: scores alternate a0/a1 (2 banks each);
            # filler (proj/outproj) half tiles alternate b0/b1 (1 bank
            # each) so consecutive tiles never WAR-serialize on a drain;
            # rotate through all four in pure-proj stretches ---
            rrc = [0]
            fc_ = [0]
            sc_ = [0]

            def _atile(i):
                return psS.tile([P, 1024], F32, tag=f"a{i}", name=f"a{i}")

            def _btile(i):
                return psS.tile([P, W], F32, tag=f"b{i}", name=f"b{i}")

            def strip(role):
                # returns [P, 1024] for scores, [P, W] for fillers
                if role == "s":
                    i = sc_[0] % 2
                    sc_[0] += 1
                    return _atile(i)
                if role == "rr":
                    i = rrc[0] % 4
                    rrc[0] += 1
                    if i < 2:
                        return _atile(i)[:, 0:W]
                    return _btile(i - 2)
                i = fc_[0] % 2
                fc_[0] += 1
                return _btile(i)

            # --- constants (already bf16/pre-arranged from host) ---
            tri_bf = cst.tile([P, P], BF16, tag="tri")
            b3_sb = cst.tile([P, 4 * OC], F32, tag="bias3")
            bq_sb = b3_sb[:, 0:OC]
            bk_sb = b3_sb[:, OC:2 * OC]
            bv_sb = b3_sb[0:HD, 2 * OC:2 * OC + NHC]
            w_sbs = {}
            for name in ("wq", "wk", "wv"):
                w_sbs[name] = cst.tile([P, FC, DC], BF16, tag=name, name=name)
            wo_sb = cst.tile([P, OC, D], BF16, tag="wo")

            # --- per-window tiles ---
            xT_w, qT_w, kT_w, v_w, ctx_w = [], [], [], [], []
            for j in range(NWIN):
                xT_w.append(cst.tile([P, FC, W], BF16, tag=f"xT{j % 2}",
                                     name=f"xT{j % 2}"))
                qT_w.append(cst.tile([P, OC, W], BF16, tag=f"qT{j}",
                                     name=f"qT{j}"))
                kT_w.append(cst.tile([P, OC, W], BF16, tag=f"kT{j}",
                                     name=f"kT{j}"))
                v_w.append(cst.tile([P, WT, NHC, HD + 1], BF16, tag=f"v{j}",
                                    name=f"v{j}"))
                ctx_w.append(cst.tile([P, NPAIR, W], BF16, tag=f"ctx{j}",
                                      name=f"ctx{j}"))
                nc.vector.memset(v_w[j][:, :, :, HD:HD + 1], 1.0)
            # ctx staging incl. the rowsum ones-row at partition HD: one
            # [65, W] DVE copy drains ctx and rowsum together; the rowsum
            # row is then reciprocal'd in place
            stgw_w = [cst.tile([HD + 1, NHC, W], F32, tag=f"stgw{j % 2}",
                               name=f"stgw{j % 2}") for j in range(NWIN)]
            rsw_live = {}

            def emit_weights_and_x():
                nc.sync.dma_start(xT_w[0][:], xt[:, :, 0:W])
                nc.gpsimd.dma_start(w_sbs["wq"][:], wq[:])
                nc.sync.dma_start(w_sbs["wk"][:], wk[:])
                nc.gpsimd.dma_start(w_sbs["wv"][:], wv[:])
                nc.gpsimd.dma_start(b3_sb[:], bias3[:])
                nc.gpsimd.dma_start(tri_bf[:], tri[:])
                nc.gpsimd.dma_start(xT_w[1][:], xt[:, :, W:2 * W])
                nc.sync.dma_start(wo_sb[:], wo[:])

            # --- filler generators: yield once per emitted matmul ---
            # frole: strip role for filler tiles.  s2 while interleaved with
            # attention chunks (scores own s0/s1); rotate-all during
            # pure-proj stretches so back-to-back tiles double-buffer.
            frole = ["f"]

            def gen_proj(j, tiles=None):
                if tiles is None:
                    tiles = ([("qh", i) for i in range(OC)]
                             + [("kh", i) for i in range(OC)]
                             + [("vh", t) for t in range(WT)])
                for kind, idx in tiles:
                    ps = strip(frole[0])
                    if kind in ("qh", "kh"):
                        dst = qT_w[j] if kind[0] == "q" else kT_w[j]
                        w_sb = w_sbs["wq" if kind[0] == "q" else "wk"]
                        b_sb = bq_sb if kind[0] == "q" else bk_sb
                        oc = idx
                        for fc in range(FC):
                            nc.tensor.matmul(
                                ps[:, 0:W],
                                w_sb[:, fc, oc * P:(oc + 1) * P],
                                xT_w[j][:, fc, :],
                                start=(fc == 0), stop=(fc == FC - 1))
                            if fc == FC - 1:
                                # drain before the last yield so a consumer
                                # emitted right after this pull sees it
                                nc.vector.tensor_scalar(
                                    dst[:, oc, :], ps[:, 0:W],
                                    b_sb[:, oc:oc + 1], None, ALU.add)
                            yield
                    else:
                        t = idx
                        for fc in range(FC):
                            nc.tensor.matmul(
                                ps[:, 0:W],
                                xT_w[j][:, fc, t * P:(t + 1) * P],
                                w_sbs["wv"][:, fc, :],
                                start=(fc == 0), stop=(fc == FC - 1))
                            if fc == FC - 1:
                                dv = ps[:, 0:W].rearrange("p (h n) -> p h n",
                                                          h=NHC)
                                nc.vector.tensor_copy(v_w[j][:, t, :, 0:HD],
                                                      dv)
                            yield

            def gen_outproj(j):
                for t in range(WT):
                    tokc = j * WT + t
                    for nb in range(2):
                        ps = strip(frole[0])
                        for pr in range(NPAIR):
                            nc.tensor.matmul(
                                ps[:, 0:W],
                                ctx_w[j][:, pr, t * P:(t + 1) * P],
                                wo_sb[:, pr, nb * 512:(nb + 1) * 512],
                                start=(pr == 0), stop=(pr == NPAIR - 1))
                            if pr == NPAIR - 1:
                                ost = stg.tile([P, W], F32, tag="ostage")
                                nc.vector.tensor_copy(ost[:], ps[:, 0:W])
                                nc.gpsimd.dma_start(
                                    out[tokc * P:(tokc + 1) * P,
                                        nb * W:(nb + 1) * W], ost[:])
                            yield

            fillq = deque()  # entries: (kind, window, generator)

            def pull(n):
                while n > 0 and fillq:
                    try:
                        next(fillq[0][2])
                        n -= 1
                    except StopIteration:
                        fillq.popleft()

            def flush_proj(j):
                frole[0] = "rr"
                while fillq and fillq[0][0] == "proj" and fillq[0][1] <= j:
                    for _ in fillq[0][2]:
                        pass
                    fillq.popleft()
                frole[0] = "f"

            def drain_fillers():
                frole[0] = "rr"
                while fillq:
                    for _ in fillq[0][2]:
                        pass
                    fillq.popleft()

            scc = [0]

            def emit_attn_pair(j, p):
                skc_hi = WT * (j + 1)
                ctx0 = psC.tile([P, W], F32, tag="c0", name="ctx0")
                ctx1 = psC.tile([P, W], F32, tag="c1", name="ctx1")

                def emit_ctx(skc, ptv, vs):
                    jk, tk = divmod(skc, WT)
                    st0 = (skc == 0)
                    sp0 = (skc == skc_hi - 1)
                    nc.tensor.matmul(ctx0[0:HD + 1, vs:W],
                                     v_w[jk][:, tk, 2 * p, :],
                                     ptv[:, 0, vs:W], start=st0, stop=sp0)
                    nc.tensor.matmul(ctx1[0:HD + 1, vs:W],
                                     v_w[jk][:, tk, 2 * p + 1, :],
                                     ptv[:, 1, vs:W], start=st0, stop=sp0)

                prev = None
                for skc in range(skc_hi):
                    jk, tk = divmod(skc, WT)
                    rel = skc * P - j * W
                    vs = max(rel, 0)
                    sp = strip("s")
                    scc[0] += 1
                    spv = sp.rearrange("p (h n) -> p h n", h=2)
                    nc.tensor.matmul(sp[:, vs:W],
                                     kT_w[jk][0:HD, p, tk * P:(tk + 1) * P],
                                     qT_w[j][0:HD, p, vs:W],
                                     start=True, stop=True)
                    nc.tensor.matmul(sp[:, W + vs:2 * W],
                                     kT_w[jk][HD:P, p, tk * P:(tk + 1) * P],
                                     qT_w[j][HD:P, p, vs:W],
                                     start=True, stop=True)
                    pt = ptp.tile([P, 1024], BF16, tag="pt", name="pt")
                    ptv = pt.rearrange("p (h n) -> p h n", h=2)
                    nc.scalar.activation(ptv[:, :, vs:W], spv[:, :, vs:W],
                                         AF.Exp, scale=SCALE)
                    if rel >= 0:
                        nc.vector.tensor_tensor(
                            ptv[:, :, rel:rel + P], ptv[:, :, rel:rel + P],
                            tri_bf[:, None, :].to_broadcast([P, 2, P]),
                            ALU.mult)
                    # software-pipeline: previous chunk's ctx lands after
                    # this chunk's scores, so it never waits on its exp
                    if prev is not None:
                        emit_ctx(*prev)
                    prev = (skc, ptv, vs)
                    pull((6, 2, 1, 1)[j])
                emit_ctx(*prev)
                # drain ctx + rowsum row in one [65, W] DVE copy; lift the
                # rowsum rows to partitions 2p..2p+1 with 1-descriptor DMAs.
                # The reciprocal+broadcast and the normalization multiplies
                # are deferred so they never wait inside an engine queue.
                rsw = sml.tile([2, W], F32, tag=f"rswp{p % 2}", name="rswp",
                               bufs=1)
                rsw_live[(j, p)] = rsw
                for h, ctxp in ((0, ctx0), (1, ctx1)):
                    i = 2 * p + h
                    nc.vector.tensor_copy(stgw_w[j][:, i, :],
                                          ctxp[0:HD + 1, :])
                    nc.gpsimd.dma_start(rsw[h:h + 1, :],
                                         stgw_w[j][HD:HD + 1, i, :])
                tick[0] += 1
                run_due()
                # the exp tail of this pair still grinds on ACT; give the
                # PE a burst of fillers so it does not idle into the next
                # pair's first ctx (which waits on the drains above)
                pull((4, 8, 12, 14)[j])
                lag1 = 2 if j < NWIN - 1 else 1
                lag2 = 4 if j < NWIN - 1 else 2
                defer(lag1, recip_bc, j, p)
                defer(lag2, norm_mults, j, p)
                if p == NPAIR - 1:
                    defer(lag2, finish_window, j)

            # --- deferred actions keyed on the global pair tick ---
            tick = [0]
            deferred = deque()

            def defer(lag, fn, *args):
                deferred.append((tick[0] + lag, fn, args))

            def run_due():
                while deferred and deferred[0][0] <= tick[0]:
                    _, fn, args = deferred.popleft()
                    fn(*args)

            def flush_deferred():
                while deferred:
                    _, fn, args = deferred.popleft()
                    fn(*args)

            shw_w = [sml.tile([HD, NPAIR, W], BF16, tag="shw", name="shw",
                              bufs=1) for _ in range(1)]
            bc_live = {}

            def recip_bc(j, p):
                rcf = sml.tile([2, W], F32, tag=f"rcfp{p % 2}", name="rcfp",
                               bufs=1)
                nc.vector.reciprocal_approx_fast(
                    rcf[:], rsw_live.pop((j, p))[:])
                for h in range(2):
                    i = 2 * p + h
                    bc = sml.tile([HD, W], F32, tag=f"bc{i % 4}", name="bc",
                                  bufs=1)
                    nc.sync.dma_start(
                        bc[:], rcf[h:h + 1, None, :].to_broadcast(
                            [1, HD, W]))
                    bc_live[(j, p, h)] = bc

            def norm_mults(j, p):
                for h in range(2):
                    i = 2 * p + h
                    dst = (ctx_w[j][0:HD, p, :] if h == 0
                           else shw_w[0][:, p, :])
                    nc.vector.tensor_tensor(dst, stgw_w[j][0:HD, i, :],
                                            bc_live.pop((j, p, h))[:],
                                            ALU.mult)
                    if with_bv:
                        nc.vector.tensor_scalar(
                            dst, dst, bv_sb[:, i:i + 1], None, ALU.add)
                q = nc.sync if p % 2 == 0 else nc.gpsimd
                q.dma_start(ctx_w[j][HD:P, p, :], shw_w[0][:, p, :])

            def finish_window(j):
                fillq.append(("outproj", j, gen_outproj(j)))

            # --- schedule ---
            emit_weights_and_x()
            frole[0] = "rr"
            for _ in gen_proj(0, tiles=[("qh", 0), ("kh", 0), ("vh", 0),
                                        ("vh", 1)]):
                pass
            frole[0] = "f"
            fillq.append(("proj", 0, gen_proj(0, tiles=[("vh", 2), ("vh", 3),
                                                        ("qh", 1), ("kh", 1),
                                                        ("qh", 2), ("kh", 2),
                                                        ("qh", 3),
                                                        ("kh", 3)])))
            for j in range(1, NWIN):
                fillq.append(("proj", j, gen_proj(j)))
            for j in range(NWIN):
                for p in range(NPAIR):
                    if p > 0 or j > 0:
                        flush_proj(j)
                    if p == 1 and j + 2 < NWIN:
                        q = nc.gpsimd if j % 2 else nc.sync
                        q.dma_start(xT_w[j + 2][:],
                                    xt[:, :, (j + 2) * W:(j + 3) * W])
                    emit_attn_pair(j, p)
            flush_deferred()
            drain_fillers()

    nc.compile()
    return nc


def make_in_maps(x, Wq, bq, Wk, bk, Wv, bv, Wo):
    BF = ml_dtypes.bfloat16
    # tri[p, f] = 1 where f >= p (keep key p for query f within a diag block)
    tri = np.triu(np.ones((P, P), dtype=np.float32)).astype(BF)
    in_maps = []
    for c in range(8):
        b, g = c // 2, c % 2
        sl = slice(g * DC, (g + 1) * DC)
        def warr(w):
            return np.ascontiguousarray(
                w.reshape(-1, P, w.shape[1]).transpose(1, 0, 2)).astype(BF)
        bias3 = np.zeros((P, 4 * OC), np.float32)
        bias3[:, 0:OC] = bq[sl].reshape(OC, P).T
        bias3[:, OC:2 * OC] = bk[sl].reshape(OC, P).T
        bias3[0:HD, 2 * OC:2 * OC + NHC] = bv[sl].reshape(NHC, HD).T
        xtb = np.ascontiguousarray(
            x[b].T.reshape(FC, P, -1).transpose(1, 0, 2)).astype(BF)
        in_maps.append({
            "xt": xtb,
            "wq": warr(Wq[:, sl]),
            "wk": warr(Wk[:, sl]),
            "wv": warr(Wv[:, sl]),
            "wo": warr(Wo[sl, :]),
            "bias3": np.ascontiguousarray(bias3.astype(np.float32)),
            "tri": tri,
        })
    return in_maps


_NC_CACHE = {}


def kernel(x, Wq, bq, Wk, bk, Wv, bv, Wo, bo):
    x = np.asarray(x, dtype=np.float32)
    args = [np.asarray(a, dtype=np.float32)
            for a in (Wq, bq, Wk, bk, Wv, bv, Wo, bo)]
    Wq, bq, Wk, bk, Wv, bv, Wo, bo = args
    key = ("nc", x.shape[1], bool(np.any(bv)))
    if key not in _NC_CACHE:
        _NC_CACHE[key] = build_nc(S=x.shape[1], num_devices=8,
                                  with_bv=bool(np.any(bv)))
    nc = _NC_CACHE[key]
    in_maps = make_in_maps(x, Wq, bq, Wk, bk, Wv, bv, Wo)
    res = run_bass_kernel_spmd(nc, in_maps, core_ids=list(range(8)))
    B = x.shape[0]
    out = np.empty_like(x)
    for b in range(B):
        out[b] = res.results[2 * b]["out"] + res.results[2 * b + 1]["out"] + bo
    return out
